# revision 31
# baseline (speedup 1.0000x reference)
"""Multi-head attention (B=2, S=2048, E=1024, H=16) on 8 Trainium2 NeuronCores.

Zero-collective data-parallel sharding: core c owns batch b=c//4 and query
tokens q0=(c%4)*512 .. q0+512 of that batch. Each core projects K/V for the
whole batch (replicated across the 4 cores sharing the batch), Q for its own
512 tokens, runs attention for all 16 heads, and computes the full output
projection for its 512-token slice. No collectives / cross-core sync, so each
core's NEFF time is its own work only (immune to peer dispatch skew).

Keys are fed to each core pre-rotated (np.roll(x[b], -q0)) so the query slice
is always rows 0:512 of the core's x — one SPMD program for all cores.
Softmax + context are permutation-invariant over keys, so rotation is safe.

All matmuls run as float32r (relaxed fp32 multiply, 1 cycle/row on the PE)
with fp32 PSUM accumulation.
"""

import sys

if "/opt/trn_rl_repo" not in sys.path:
    sys.path.insert(0, "/opt/trn_rl_repo")

import numpy as np

B, S, E, H, DH = 2, 2048, 1024, 16, 64
NCORES = 8
NQ = 512           # query tokens per core
S2 = S             # key/value tokens per core (whole batch)
HB = 8             # head-blocks of 128 cols (2 heads) each
NJ = S2 // 128     # 16 key tiles

_CACHE = {}


def _build(debug=False):
    from contextlib import ExitStack

    import concourse.bacc as bacc
    import concourse.bass as bass
    import concourse.mybir as mybir
    import concourse.tile as tile
    from concourse.masks import make_identity

    f32 = mybir.dt.float32
    f32r = mybir.dt.float32r
    bf16 = mybir.dt.bfloat16

    nc = bacc.Bacc("TRN2", num_devices=NCORES)

    xt_d = nc.declare_dram_parameter("xt", [E, S2], bf16, isOutput=False)
    wq_d = nc.declare_dram_parameter("wq", [E, E], bf16, isOutput=False)
    wk_d = nc.declare_dram_parameter("wk", [E, E], bf16, isOutput=False)
    wv_d = nc.declare_dram_parameter("wv", [E, E], bf16, isOutput=False)
    wo_d = nc.declare_dram_parameter("wo", [E, E], bf16, isOutput=False)
    bq_d = nc.declare_dram_parameter("bq", [E], f32, isOutput=False)
    bk_d = nc.declare_dram_parameter("bk", [E], f32, isOutput=False)
    bv_d = nc.declare_dram_parameter("bv", [E], f32, isOutput=False)
    bo_d = nc.declare_dram_parameter("bo", [E], f32, isOutput=False)
    out_d = nc.declare_dram_parameter("out", [NQ, E], f32, isOutput=True)

    with tile.TileContext(nc) as tc, ExitStack() as ctx:
        singles = ctx.enter_context(tc.tile_pool(name="singles", bufs=1))

        ident = singles.tile([128, 128], f32)
        make_identity(nc, ident)
        identb = singles.tile([128, 128], bf16, tag="identb")
        nc.vector.tensor_copy(out=identb, in_=ident)

        # constants for softmax-denominator broadcast + head-pair stacking
        cb64 = singles.tile([128, 64], f32r, tag="cb64")  # all 1/64
        nc.vector.tensor_scalar(
            out=cb64, in0=ident[:, 0:64], scalar1=0.0, scalar2=1.0 / 64.0,
            op0=mybir.AluOpType.mult, op1=mybir.AluOpType.add,
        )
        # stk[:, h, :]: [64, 128] with I64 in cols h*64..h*64+64
        stk = singles.tile([64, 2, 128], bf16, tag="stk")
        for h in range(2):
            nc.vector.tensor_scalar(
                out=stk[:, h, :], in0=ident[0:64, :],
                scalar1=0.0, scalar2=0.0,
                op0=mybir.AluOpType.mult, op1=mybir.AluOpType.add,
            )
        nc.vector.tensor_copy(out=stk[:, 0, 0:64], in_=ident[0:64, 0:64])
        nc.vector.tensor_copy(out=stk[:, 1, 64:128], in_=ident[0:64, 0:64])

        # biases: [128, 3, 8] (q/k/v, col = head-block)
        bias_sb = singles.tile([128, 3, 8], f32, tag="bias")
        for i, b_d in enumerate((bq_d, bk_d, bv_d)):
            nc.sync.dma_start(
                out=bias_sb[:, i, :], in_=b_d.ap().rearrange("(o p) -> p o", p=128)
            )
        bqs, bks, bvs = bias_sb[:, 0, :], bias_sb[:, 1, :], bias_sb[:, 2, :]
        # persistent activations
        xT = singles.tile([128, 8, S2], bf16, tag="xT")          # x^T, 8 MB
        ctxT_sb = singles.tile([128, HB, NQ], bf16, tag="ctxT")  # ctx^T, 2 MB
        v_ones = singles.tile([128, NJ, 2, 128], bf16, tag="vones")  # 2 MB

        # hb0 weights + wo lead the DMA queue so the first proj matmul and
        # the phase-C weights are never transfer-gated; xT chunks follow.
        wearly = ctx.enter_context(tc.tile_pool(name="wearly", bufs=1))
        w0q = wearly.tile([128, 8, 128], bf16, tag="w0q")
        w0k = wearly.tile([128, 8, 128], bf16, tag="w0k")
        w0v = wearly.tile([128, 8, 128], bf16, tag="w0v")
        for w_s, w_d in ((w0q, wq_d), (w0k, wk_d), (w0v, wv_d)):
            nc.sync.dma_start(
                out=w_s, in_=w_d.ap()[:, 0:128].rearrange("(o p) c -> p o c", p=128)
            )

        # --- phase A: load pre-transposed x (host ships x[b].T) ---
        for tchunk in range(4):
            tsl = slice(tchunk * 512, (tchunk + 1) * 512)
            nc.sync.dma_start(
                out=xT[:, :, tsl],
                in_=xt_d.ap()[:, tsl].rearrange("(o p) t -> p o t", p=128),
            )

        wo_sb = wearly.tile([128, 8, E], bf16, tag="wo")  # 2 MB
        for eh in range(2):
            nc.sync.dma_start(
                out=wo_sb[:, :, eh * 512:(eh + 1) * 512],
                in_=wo_d.ap()[:, eh * 512:(eh + 1) * 512].rearrange(
                    "(o p) e -> p o e", p=128
                ),
            )
        bo_bc = wearly.tile([128, E], f32, tag="bo")
        nc.gpsimd.dma_start(
            out=bo_bc, in_=bo_d.ap().unsqueeze(0).broadcast_to([128, E])
        )

        # ones planes (cols 64:128) for the PE row-sum trick; v planes are
        # rewritten per head-block. memset can't target f32-for-PE tiles, so
        # fill via x*0+1 from the (now initialized) xT.
        nc.vector.tensor_scalar(
            out=v_ones.rearrange("p j h c -> p (j h) c")[:, :, 64:128],
            in0=xT[:, 0, :].rearrange("p (a b) -> p a b", a=32),
            scalar1=0.0, scalar2=1.0,
            op0=mybir.AluOpType.mult, op1=mybir.AluOpType.add,
        )

        # --- phase B: per head-block: project K/V/Q, attention, normalize ---
        with (
            tc.tile_pool(name="wreal", bufs=2) as wreal,
            tc.tile_pool(name="kpool", bufs=2) as kpool,
            tc.tile_pool(name="vpool", bufs=1) as vpool,
            tc.tile_pool(name="qpool", bufs=2) as qpool,
            tc.tile_pool(name="expool", bufs=3) as expool,
            tc.tile_pool(name="dvp", bufs=2) as dvp,
            tc.tile_pool(name="projps", bufs=2, space="PSUM") as projps,
            tc.tile_pool(name="stps", bufs=2, space="PSUM") as stps,
            tc.tile_pool(name="ctxps", bufs=2, space="PSUM") as ctxps,
        ):
            for hb in range(HB):
                csl = slice(hb * 128, (hb + 1) * 128)
                if hb == 0:
                    wq_s, wk_s, wv_s = w0q, w0k, w0v
                else:
                    wq_s = wreal.tile([128, 8, 128], bf16, tag="wq")
                    wk_s = wreal.tile([128, 8, 128], bf16, tag="wk")
                    wv_s = wreal.tile([128, 8, 128], bf16, tag="wv")
                    for w_s, w_d in ((wq_s, wq_d), (wk_s, wk_d), (wv_s, wv_d)):
                        nc.sync.dma_start(
                            out=w_s,
                            in_=w_d.ap()[:, csl].rearrange("(o p) c -> p o c", p=128),
                        )

                # K^T / V^T over all 2048 tokens, Q^T over own 512 tokens
                kT = kpool.tile([128, S2], bf16, tag="kT")
                vT = vpool.tile([128, S2], bf16, tag="vT")
                qT = qpool.tile([128, NQ], bf16, tag="qT")
                for w_s, bias, dstT, nts in (
                    (wk_s, bks, kT, 4),
                    (wv_s, bvs, vT, 4),
                    (wq_s, bqs, qT, 1),
                ):
                    for ts4 in range(nts):
                        t4 = slice(ts4 * 512, (ts4 + 1) * 512)
                        ps = projps.tile([128, 512], f32, tag="proj")
                        for ec in range(8):
                            nc.tensor.matmul(
                                ps, w_s[:, ec, :], xT[:, ec, t4],
                                start=(ec == 0), stop=(ec == 7),
                            )
                        nc.vector.tensor_scalar_add(
                            out=dstT[:, t4], in0=ps, scalar1=bias[:, hb:hb + 1]
                        )

                # V natural layout into v_ones planes
                for j in range(NJ):
                    jsl = slice(j * 128, (j + 1) * 128)
                    ps_v = projps.tile([128, 128], bf16, tag="proj")
                    nc.tensor.transpose(ps_v, vT[:, jsl], identb)
                    nc.vector.tensor_copy(out=v_ones[:, j, 0, 0:64], in_=ps_v[:, 0:64])
                    nc.vector.tensor_copy(out=v_ones[:, j, 1, 0:64], in_=ps_v[:, 64:128])

                # attention, h0/h1 interleaved per key tile: the two score
                # matmuls use disjoint PE row-halves (tile_position from the
                # 64-partition base), so each LDWEIGHTS overlaps the other
                # half's matmul. One exp covers both heads' scores.
                ctx0 = ctxps.tile([128, NQ], f32, tag="ctx")
                ctx1 = ctxps.tile([128, NQ], f32, tag="ctx")
                ctx_ps = [ctx0, ctx1]
                for j in range(NJ):
                    jsl = slice(j * 128, (j + 1) * 128)
                    st2 = stps.tile([128, 2, NQ], f32, tag="st")
                    for h in range(2):
                        hr = slice(64 * h, 64 * h + 64)
                        nc.tensor.matmul(
                            st2[:, h, :], kT[hr, jsl], qT[hr, :],
                            start=True, stop=True,
                        )
                    ex2 = expool.tile([128, 2, NQ], bf16, tag="ex")
                    nc.scalar.activation(
                        out=ex2.rearrange("p a b -> p (a b)"),
                        in_=st2.rearrange("p a b -> p (a b)"),
                        func=mybir.ActivationFunctionType.Exp, scale=0.125,
                    )
                    for h in range(2):
                        nc.tensor.matmul(
                            ctx_ps[h], v_ones[:, j, h, :], ex2[:, h, :],
                            start=(j == 0), stop=(j == NJ - 1),
                        )

                # normalize each head at partition offset 0 (exp row-sums are
                # on partitions 64:128 -> PE-broadcast down via cb64), then
                # stack the normalized pair into ctxT_sb[:, hb, :] on the PE.
                cn = [None, None]
                for h in range(2):
                    l_sb = dvp.tile([128, NQ], f32r, tag=f"lsb{h}")
                    nc.vector.tensor_copy(
                        out=l_sb[64:128, :], in_=ctx_ps[h][64:128, :]
                    )
                    lr = stps.tile([64, NQ], f32, tag="st")
                    nc.tensor.matmul(
                        lr, cb64[64:128, :], l_sb[64:128, :], start=True, stop=True
                    )
                    recip = dvp.tile([64, NQ], f32, tag=f"recip{h}")
                    nc.vector.reciprocal_approx_fast(out=recip, in_=lr)
                    cnh = dvp.tile([64, NQ], bf16, tag=f"cn{h}")
                    nc.vector.tensor_mul(
                        out=cnh, in0=ctx_ps[h][0:64, :], in1=recip
                    )
                    cn[h] = cnh
                pair_ps = stps.tile([128, NQ], f32, tag="st")
                nc.tensor.matmul(pair_ps, stk[:, 0, :], cn[0], start=True, stop=False)
                nc.tensor.matmul(pair_ps, stk[:, 1, :], cn[1], start=False, stop=True)
                nc.vector.tensor_copy(out=ctxT_sb[:, hb, :], in_=pair_ps)

        # --- phase C: output projection for own 512-token slice ---
        with (
            tc.tile_pool(name="ph4", bufs=3) as ph4,
            tc.tile_pool(name="ph4ps", bufs=2, space="PSUM") as ph4ps,
        ):
            for tt in range(4):
                for eh in range(2):
                    esl = slice(eh * 512, (eh + 1) * 512)
                    ps_o = ph4ps.tile([128, 512], f32, tag="o")
                    for hb in range(HB):
                        nc.tensor.matmul(
                            ps_o,
                            ctxT_sb[:, hb, tt * 128:(tt + 1) * 128],
                            wo_sb[:, hb, esl],
                            start=(hb == 0), stop=(hb == HB - 1),
                        )
                    o_sb = ph4.tile([128, 512], f32, tag="osb")
                    nc.vector.tensor_add(out=o_sb, in0=ps_o, in1=bo_bc[:, esl])
                    nc.sync.dma_start(
                        out=out_d.ap()[tt * 128:(tt + 1) * 128, esl], in_=o_sb
                    )

    nc.finalize()
    return nc


def _get_nc():
    import os
    debug = bool(int(os.environ.get("MHA_DEBUG", "0")))
    key = ("nc", debug)
    if key not in _CACHE:
        _CACHE[key] = _build(debug)
    return _CACHE[key]


def kernel(x, Wq, bq, Wk, bk, Wv, bv, Wo, bo, **_ignored):
    import ml_dtypes
    from concourse.bass_utils import run_bass_kernel_spmd

    bf = ml_dtypes.bfloat16
    x = np.asarray(x, dtype=np.float32).astype(bf)
    Wq = np.ascontiguousarray(np.asarray(Wq, dtype=np.float32).astype(bf))
    Wk = np.ascontiguousarray(np.asarray(Wk, dtype=np.float32).astype(bf))
    Wv = np.ascontiguousarray(np.asarray(Wv, dtype=np.float32).astype(bf))
    Wo = np.ascontiguousarray(np.asarray(Wo, dtype=np.float32).astype(bf))
    bq = np.ascontiguousarray(np.asarray(bq, dtype=np.float32))
    bk = np.ascontiguousarray(np.asarray(bk, dtype=np.float32))
    bv = np.ascontiguousarray(np.asarray(bv, dtype=np.float32))
    bo = np.ascontiguousarray(np.asarray(bo, dtype=np.float32))

    in_maps = []
    xts = {}
    for c in range(NCORES):
        b, q0 = c // 4, (c % 4) * NQ
        if (b, q0) not in xts:
            xb = x[b] if q0 == 0 else np.roll(x[b], -q0, axis=0)
            xts[(b, q0)] = np.ascontiguousarray(xb.T)
        in_maps.append(
            {
                "xt": xts[(b, q0)],
                "wq": Wq, "wk": Wk, "wv": Wv, "wo": Wo,
                "bq": bq, "bk": bk, "bv": bv, "bo": bo,
            }
        )

    nc = _get_nc()
    import os

    trace = bool(int(os.environ.get("MHA_TRACE", "0")))
    res = run_bass_kernel_spmd(
        nc, in_maps, core_ids=list(range(NCORES)), trace=trace
    )
    if trace:
        _CACHE["last_results"] = res
    _CACHE["res"] = res
    out = np.empty((B, S, E), dtype=np.float32)
    for c in range(NCORES):
        b, q0 = c // 4, (c % 4) * NQ
        out[b, q0:q0 + NQ] = res.results[c]["out"]
    return out
